# revision 3
# baseline (speedup 1.0000x reference)
"""Trainium2 Bass kernel for nn_NeuralAttention (MLP-scored attention).

Math (per head h, batch 1, n=512, dh=64, P=32):
  qkv = x @ Wqkv^T, split 'b n (d k h) -> k b h n d'
  qp = q@Wq^T+bq ; kp = k@Wk^T+bk
  a  = qp@W1q^T  ; c = kp@W1k^T          (W1 = [W1q | W1k])
  h1 = relu(a_i + c_j + b1)              # [n, n, 32]
  h2 = relu(h1 @ W2^T + b2)              # [n, n, 16]
  s  = h2 @ W3^T (+ b3, drops in softmax)
  attn = softmax(causal(s)) ; out = attn @ v ; y = out @ Wout^T

Key algebra used:
  a = q @ (W1q Wq)^T + W1q bq  => Aq = W1q@Wq, fold consts into one
  per-partition constant s1const = W1q bq + W1k bk + b1.

Sharding: 16 heads over 8 cores (2 heads/core), Wout row-parallel;
host sums the 8 partial [1024, 512] outputs (unshard of row-parallel
layout) and transposes.

On-device layout ("j on partitions"): scores^T[j, i] computed in
j-tiles of 128 with only the causal i-suffix evaluated.  Stage-1
relu(a_i + c_j) runs on DVE as tensor_scalar (bf16, 4x mode) with the
per-partition scalar = packed c columns; stage-2 is a block-diagonal
(4x) W2 matmul; stage-3 scatters 8-row score strips into 32-aligned
PSUM blocks via zero-padded W3 weights accumulated 4 matmuls/block.
Softmax needs no max-subtraction (|s| < ~1) and the denominator comes
free from a ones-column appended to V in the attn@V matmul.
"""

import sys

sys.path.insert(0, "/opt/trn_rl_repo")

from contextlib import ExitStack

import ml_dtypes
import numpy as np

import concourse.bass as bass
import concourse.tile as tile
from concourse import bacc, mybir
from concourse.bass_utils import run_bass_kernel_spmd

F32 = mybir.dt.float32
BF16 = mybir.dt.bfloat16
AF = mybir.ActivationFunctionType
ALU = mybir.AluOpType

B, N, DIM = 1, 512, 1024
HEADS, DH = 16, 64
P, P2 = 32, 16
N_CORES = 8
HPC = HEADS // N_CORES  # heads per core = 2
NT = N // 128           # j tiles = 4
KT = DIM // 128         # contraction tiles for projections = 8


# ---------------------------------------------------------------- program ---

def build_program(repeat: int = 1):
    nc = bacc.Bacc("TRN2", target_bir_lowering=False, debug=False,
                   num_devices=N_CORES)

    d = {}
    def din(name, shape, dt):
        d[name] = nc.dram_tensor(name, shape, dt, kind="ExternalInput").ap()
        return d[name]

    xT_d = din("xT", [DIM, N], F32)            # x transposed
    wqkT_d = din("wqkT", [DIM, 4 * DH], BF16)  # [q_h0 q_h1 k_h0 k_h1] lhsT
    wvT_d = din("wvT", [DIM, HPC * DH], F32)   # v rhs (both heads)
    aqrep_d = din("aqrep", [DH, 128], BF16)    # Aq^T replicated 4x (cols)
    akT_d = din("akT", [DH, P], BF16)          # Ak^T
    s1c_d = din("s1c", [128, 1], F32)          # (W1q bq + W1k bk + b1) rep 4x
    w2b_d = din("w2b", [128, 64], BF16)        # blockdiag4(W2^T)
    b2r_d = din("b2r", [128, 1], F32)          # b2 tiled 8x
    w3s_d = din("w3s", [4, 128, P], BF16)      # scatter W3 blocks
    tri_d = din("tri", [128, 128], F32)        # lower-tri 0/1 (i>=j valid)
    woutT_d = din("woutT", [HPC, DH, DIM], F32)  # per-head Wout slice lhsT

    outT_d = nc.dram_tensor("outT", [DIM, N], F32, kind="ExternalOutput").ap()

    with tile.TileContext(nc) as tc, ExitStack() as ctx:
        cst = ctx.enter_context(tc.tile_pool(name="cst", bufs=1))

        # --- load constants / x ---
        xT32 = []
        xT16 = []
        xv = xT_d.rearrange("(a p) n -> a p n", p=128)
        for kk in range(KT):
            t32 = cst.tile([128, N], F32, tag=f"xT32_{kk}")
            nc.sync.dma_start(t32[:], xv[kk])
            xT32.append(t32)
            t16 = cst.tile([128, N], BF16, tag=f"xT16_{kk}")
            nc.vector.tensor_copy(t16[:], t32[:])
            xT16.append(t16)
        wqk = []
        wqkv_v = wqkT_d.rearrange("(a p) m -> a p m", p=128)
        for kk in range(KT):
            t = cst.tile([128, 4 * DH], BF16, tag=f"wqk_{kk}")
            nc.sync.dma_start(t[:], wqkv_v[kk])
            wqk.append(t)
        wv = []
        wv_v = wvT_d.rearrange("(a p) m -> a p m", p=128)
        for kk in range(KT):
            t = cst.tile([128, HPC * DH], F32, tag=f"wv_{kk}")
            nc.sync.dma_start(t[:], wv_v[kk])
            wv.append(t)
        aqrep = cst.tile([DH, 128], BF16, tag="aqrep")
        nc.sync.dma_start(aqrep[:], aqrep_d[:])
        akT = cst.tile([DH, P], BF16, tag="akT")
        nc.sync.dma_start(akT[:], akT_d[:])
        s1c = cst.tile([128, 1], F32, tag="s1c")
        nc.sync.dma_start(s1c[:], s1c_d[:])
        w2b = cst.tile([128, 64], BF16, tag="w2b")
        nc.sync.dma_start(w2b[:], w2b_d[:])
        b2r = cst.tile([128, 1], F32, tag="b2r")
        nc.sync.dma_start(b2r[:], b2r_d[:])
        w3s = []
        for bb in range(4):
            t = cst.tile([128, P], BF16, tag=f"w3s_{bb}")
            nc.sync.dma_start(t[:], w3s_d[bb])
            w3s.append(t)
        tri = cst.tile([128, 128], F32, tag="tri")
        nc.sync.dma_start(tri[:], tri_d[:])
        woutT = []
        for h in range(HPC):
            t = cst.tile([DH, DIM], F32, tag=f"woutT_{h}")
            nc.sync.dma_start(t[:], woutT_d[h])
            woutT.append(t)

        for rep in range(repeat):
            _body(nc, tc, ctx, rep, xT32, xT16, wqk, wv, aqrep, akT, s1c,
                  w2b, b2r, w3s, tri, woutT, outT_d)

    nc.compile()
    return nc


def _body(nc, tc, ctx, rep, xT32, xT16, wqk, wv, aqrep, akT, s1c, w2b, b2r,
          w3s, tri, woutT, outT_d):
    r = f"r{rep}"
    cst2 = ctx.enter_context(tc.tile_pool(name=f"cst2_{r}", bufs=1))

    # ---------------- P1: q/k projections -> q16/k16 [64, N] bf16 ----------
    qk16 = []  # [q_h0, q_h1, k_h0, k_h1]
    with tc.tile_pool(name=f"qkps_{r}", bufs=2, space="PSUM") as qkps:
        for m in range(4):
            ps = qkps.tile([DH, N], F32, tag="qk")
            for kk in range(KT):
                nc.tensor.matmul(ps[:, :], wqk[kk][:, m * DH:(m + 1) * DH],
                                 xT16[kk][:, :],
                                 start=(kk == 0), stop=(kk == KT - 1))
            sb = cst2.tile([DH, N], BF16, tag=f"qk16_{m}")
            nc.scalar.copy(sb[:], ps[:])
            qk16.append(sb)

    # ---------------- P2: v projection -> v' [128, 130] f32 per j-tile -----
    vp = cst2.tile([128, NT * 130], F32, tag="vp")
    with tc.tile_pool(name=f"vps_{r}", bufs=2, space="PSUM") as vps:
        for t in range(NT):
            ps = vps.tile([128, HPC * DH], F32, tag="v")
            for kk in range(KT):
                nc.tensor.matmul(ps[:, :], xT32[kk][:, t * 128:(t + 1) * 128],
                                 wv[kk][:, :],
                                 start=(kk == 0), stop=(kk == KT - 1))
            for h in range(HPC):
                nc.scalar.copy(vp[:, t * 130 + h * 65: t * 130 + h * 65 + DH],
                               ps[:, h * DH:(h + 1) * DH])
                nc.vector.memset(
                    vp[:, t * 130 + h * 65 + DH: t * 130 + h * 65 + 65], 1.0)

    # ---------------- P3: per-head score MLP + softmax + attn@v ------------
    out_h = []  # [64, N] f32 normalized attention output per head
    for h in range(HPC):
        hr = f"{r}h{h}"
        with tc.tile_pool(name=f"m_{hr}", bufs=1, space="PSUM") as mps, \
             tc.tile_pool(name=f"s2_{hr}", bufs=2, space="PSUM") as s2ps, \
             tc.tile_pool(name=f"sc_{hr}", bufs=1, space="PSUM") as scps, \
             tc.tile_pool(name=f"op_{hr}", bufs=1, space="PSUM") as ops, \
             tc.tile_pool(name=f"wk_{hr}", bufs=4) as wk, \
             tc.tile_pool(name=f"h2_{hr}", bufs=3) as h2p, \
             tc.tile_pool(name=f"ex_{hr}", bufs=2) as exp_pool:

            # a4 = 4x-replicated a^T (+ s1const via copy bias) [128, N] bf16
            a_ps = mps.tile([128, N], F32, tag="m")
            nc.tensor.matmul(a_ps[:, :], aqrep[:, :], qk16[h][:, :],
                             start=True, stop=True)
            a4 = cst2.tile([128, N], BF16, tag=f"a4_{h}")
            nc.scalar.activation(a4[:], a_ps[:], AF.Identity,
                                 bias=s1c[:], scale=1.0)

            # cbias[32u+p, g] = (Ak k^T)[p, 4g+u]  [128, 128] f32
            c_ps = mps.tile([128, 128], F32, tag="m")
            k_re = qk16[2 + h][:].rearrange("d (g u) -> d u g", u=4)
            for u in range(4):
                nc.tensor.matmul(c_ps[32 * u:32 * (u + 1), :], akT[:, :],
                                 k_re[:, u, :], start=True, stop=True,
                                 tile_position=(0, 32 * u))
            cb = cst2.tile([128, 128], F32, tag=f"cb_{h}")
            nc.scalar.copy(cb[:], c_ps[:])

            # out' accumulator [65, N] psum (num rows 0..64, den row 64)
            op_ps = ops.tile([65, N], F32, tag="op")

            for t in range(NT):
                L = N - t * 128
                i0 = t * 128
                sc_ps = scps.tile([128, L], F32, tag="sc")
                for m in range(0, 16, 2):
                    ps2 = s2ps.tile([128, 2 * L], F32, tag="s2")
                    for dm in range(2):
                        for v in range(2):
                            g = 32 * t + 2 * (m + dm) + v
                            h1 = wk.tile([128, L], BF16, tag="h1")
                            nc.vector.tensor_scalar(
                                h1[:], a4[:, i0:N], cb[:, g:g + 1], 0.0,
                                ALU.add, ALU.max)
                            nc.tensor.matmul(
                                ps2[64 * v:64 * (v + 1), dm * L:(dm + 1) * L],
                                w2b[:, :], h1[:], start=True, stop=True)
                    h2 = h2p.tile([128, 2 * L], BF16, tag="h2")
                    nc.scalar.activation(h2[:], ps2[:], AF.Relu,
                                         bias=b2r[:], scale=1.0)
                    for dm in range(2):
                        mm = m + dm
                        ab, bb = mm // 4, mm % 4
                        nc.tensor.matmul(
                            sc_ps[32 * ab:32 * (ab + 1), :],
                            w3s[bb][:, :], h2[:, dm * L:(dm + 1) * L],
                            start=(bb == 0), stop=(bb == 3),
                            skip_group_check=True,
                            tile_position=(0, 32 * ab))
                ex = exp_pool.tile([128, L], F32, tag="ex")
                nc.scalar.activation(ex[:], sc_ps[:], AF.Exp)
                nc.vector.tensor_mul(ex[:, 0:128], ex[:, 0:128], tri[:])
                nc.tensor.matmul(op_ps[:, i0:N],
                                 vp[:, t * 130 + h * 65: t * 130 + h * 65 + 65],
                                 ex[:], start=(t == 0), stop=(t == NT - 1),
                                 skip_group_check=True)

            # normalize: out = num * (1/den)
            num = cst2.tile([DH, N], F32, tag=f"num_{h}")
            nc.scalar.copy(num[:], op_ps[0:DH, :])
            rsb = cst2.tile([128, N], F32, tag=f"rec_{h}")
            nc.vector.reciprocal(rsb[64:65, :], op_ps[64:65, :])
            ones = cst2.tile([128, DH], F32, tag=f"ones_{h}")
            nc.vector.memset(ones[64:65, :], 1.0)
            rb_ps = mps.tile([DH, N], F32, tag="m")
            nc.tensor.matmul(rb_ps[:, :], ones[64:65, :], rsb[64:65, :],
                             start=True, stop=True)
            o = cst2.tile([DH, N], F32, tag=f"out_{h}")
            nc.vector.tensor_mul(o[:], num[:], rb_ps[:])
            out_h.append(o)

    # ---------------- P4: output projection (row-parallel Wout) ------------
    with tc.tile_pool(name=f"wo_{r}", bufs=2, space="PSUM") as wops, \
         tc.tile_pool(name=f"ob_{r}", bufs=2) as obp:
        for ot in range(KT):
            ps = wops.tile([128, N], F32, tag="wo")
            for h in range(HPC):
                nc.tensor.matmul(ps[:, :],
                                 woutT[h][:, ot * 128:(ot + 1) * 128],
                                 out_h[h][:, :],
                                 start=(h == 0), stop=(h == HPC - 1))
            ob = obp.tile([128, N], F32, tag="ob")
            nc.scalar.copy(ob[:], ps[:])
            nc.sync.dma_start(
                outT_d.rearrange("(a p) n -> a p n", p=128)[ot], ob[:])


# ---------------------------------------------------------------- host side -

def prep_inputs(x, Wqkv, Wout, Wq, bq, Wk, bk, W1, b1, W2, b2, W3, b3):
    """Build the per-core input maps (all numpy, fp32/bf16)."""
    x = np.asarray(x, np.float32).reshape(N, DIM)
    Wqkv = np.asarray(Wqkv, np.float32)
    Wout = np.asarray(Wout, np.float32)
    Wq, bq = np.asarray(Wq, np.float32), np.asarray(bq, np.float32)
    Wk, bk = np.asarray(Wk, np.float32), np.asarray(bk, np.float32)
    W1, b1 = np.asarray(W1, np.float32), np.asarray(b1, np.float32)
    W2, b2 = np.asarray(W2, np.float32), np.asarray(b2, np.float32)
    W3 = np.asarray(W3, np.float32)

    bf = lambda a: np.ascontiguousarray(a).astype(ml_dtypes.bfloat16)
    f32 = lambda a: np.ascontiguousarray(a, np.float32)

    xT = f32(x.T)                                   # [DIM, N]

    W1q, W1k = W1[:, :P], W1[:, P:]
    Aq = W1q @ Wq                                   # [32, 64]
    Ak = W1k @ Wk
    s1const = W1q @ bq + W1k @ bk + b1              # [32]
    s1c = f32(np.tile(s1const, 4)[:, None])         # [128, 1]

    aqrep = np.zeros((DH, 128), np.float32)
    for u in range(4):
        aqrep[:, 32 * u:32 * (u + 1)] = Aq.T
    akT = Ak.T                                      # [64, 32]

    w2b = np.zeros((128, 64), np.float32)
    for u in range(4):
        w2b[32 * u:32 * (u + 1), 16 * u:16 * (u + 1)] = W2.T
    b2r = f32(np.tile(b2, 8)[:, None])              # [128, 1]

    w3s = np.zeros((4, 128, P), np.float32)
    for bb in range(4):
        for v in range(2):
            for u in range(4):
                col = 8 * bb + 4 * v + u
                for q in range(P2):
                    w3s[bb, 64 * v + 16 * u + q, col] = W3[0, q]

    ii = np.arange(128)
    tri = (ii[None, :] >= ii[:, None]).astype(np.float32)  # [j, i] valid

    # per-head channel index in Wqkv output: o = d*48 + k*16 + h
    dch = np.arange(DH)
    in_maps = []
    for c in range(N_CORES):
        h0, h1 = HPC * c, HPC * c + 1
        rows_q = [dch * 48 + 0 * HEADS + h for h in (h0, h1)]
        rows_k = [dch * 48 + 1 * HEADS + h for h in (h0, h1)]
        rows_v = [dch * 48 + 2 * HEADS + h for h in (h0, h1)]
        wqkT = np.concatenate(
            [Wqkv[r] for r in rows_q + rows_k], axis=0).T     # [DIM, 256]
        wvT = np.concatenate([Wqkv[r] for r in rows_v], axis=0).T  # [DIM,128]
        woutT = np.stack(
            [Wout[:, DH * h:DH * (h + 1)].T for h in (h0, h1)])  # [2,64,DIM]
        in_maps.append({
            "xT": xT,
            "wqkT": bf(wqkT),
            "wvT": f32(wvT),
            "aqrep": bf(aqrep),
            "akT": bf(akT),
            "s1c": s1c,
            "w2b": bf(w2b),
            "b2r": b2r,
            "w3s": bf(w3s),
            "tri": f32(tri),
            "woutT": f32(woutT),
        })
    return in_maps


_PROGRAM_CACHE = {}


def _get_program(repeat=1):
    if repeat not in _PROGRAM_CACHE:
        _PROGRAM_CACHE[repeat] = build_program(repeat)
    return _PROGRAM_CACHE[repeat]


def run(in_maps, repeat=1):
    nc = _get_program(repeat)
    return run_bass_kernel_spmd(nc, in_maps, list(range(N_CORES)))


def kernel(**inputs) -> np.ndarray:
    in_maps = prep_inputs(**inputs)
    res = run(in_maps)
    acc = np.zeros((DIM, N), np.float64)
    for c in range(N_CORES):
        acc += res.results[c]["outT"].astype(np.float64)
    return np.ascontiguousarray(acc.T.astype(np.float32)).reshape(B, N, DIM)
